# revision 1
# baseline (speedup 1.0000x reference)
"""GCN layer (GCNConv + residual + BatchNorm + ReLU) on 8 Trainium2 NeuronCores.

out = relu(BN(A_hat @ x @ W + b + x)),  A_hat = D^-1/2 (A+I) D^-1/2.

Key algebra
  - Aggregation commutes with the linear transform: agg0 = A_hat @ x first,
    then one [128,128] matmul per output tile. Avoids materializing h = x@W.
  - The bias b is a per-feature constant shift; training-mode BatchNorm
    subtracts the batch mean, so b cancels exactly and is dropped.
  - Edge (u->v) contributes dinv_u*dinv_v*x_u. dinv_u is folded into a
    host-precomputed message table y = dinv*x stored as a bf16 hi+lo pair
    (exact to ~2^-17) packed in one 512-byte row; dinv_v is a per-dest
    column scale applied after aggregation.

Distribution: nodes (dest rows) are sharded across the 8 cores; each core
owns ST supertiles x 256 dest slots. A greedy balancer assigns nodes to
(core, supertile) bins so every (supertile, source-bank) edge group fits
C_B chunks of 128 edges. Per chunk: dma_gather fetches 128 message rows
(int16 indices into 4 banks of <=32k rows), a bf16 one-hot selection matrix
S[p,v] = (iota_v == dloc_p) is built on the vector engine (4x mode), and the
tensor engine accumulates msgs_hi.T @ S + msgs_lo.T @ S into fp32 PSUM.
BatchNorm statistics are accumulated per supertile with scalar-engine
accum_out and AllReduce'd across the 8 cores; pass 2 applies
relu(A*v + B) and writes the transposed output shard. The host reassembles
the full [N,128] output from the per-core shards via the slot permutation.
"""
import sys
import numpy as np
import ml_dtypes

for _p in ("/opt/trn_rl_repo", "/root/.axon_site/_ro/trn_rl_repo"):
    if _p not in sys.path:
        sys.path.append(_p)

P = 128
D = 128
NDEST = 256
NCORE = 8
NBANK = 4
BN_EPS = 1e-5


def _prepare(x, edge_index):
    N = x.shape[0]
    NV = -(-N // (NCORE * NDEST)) * NDEST
    ST = NV // NDEST
    BANK = -(-N // NBANK)
    assert BANK <= 32767

    deg = np.bincount(edge_index[1].astype(np.int64), minlength=N).astype(np.float64) + 1.0
    dinv = (1.0 / np.sqrt(deg)).astype(np.float32)

    y = x * dinv[:, None]
    y_hi = y.astype(ml_dtypes.bfloat16)
    y_lo = (y - y_hi.astype(np.float32)).astype(ml_dtypes.bfloat16)
    ypack = np.ascontiguousarray(np.concatenate([y_hi, y_lo], axis=1))
    if ypack.shape[0] < NBANK * BANK:
        pad = np.zeros((NBANK * BANK - ypack.shape[0], 2 * D), ml_dtypes.bfloat16)
        ypack = np.ascontiguousarray(np.vstack([ypack, pad]))

    src = edge_index[0].astype(np.int64)
    dst = edge_index[1].astype(np.int64)
    loop = np.arange(N, dtype=np.int64)
    src = np.concatenate([src, loop])
    dst = np.concatenate([dst, loop])

    sb = src // BANK
    cnt = np.bincount(dst * NBANK + sb, minlength=N * NBANK).reshape(N, NBANK)

    # greedy node -> (core, supertile) assignment balancing per-bank load
    nbins = NCORE * ST
    order = np.argsort(-cnt.sum(1), kind="stable")
    load = np.zeros((nbins, NBANK), np.int64)
    fill = np.zeros(nbins, np.int32)
    bin_of = np.empty(N, np.int32)
    l_of = np.empty(N, np.int32)
    BIG = 1 << 40
    for v in order:
        cand = np.max(load + cnt[v][None, :], axis=1) + np.where(fill >= NDEST, BIG, 0)
        t = int(np.argmin(cand))
        bin_of[v] = t
        l_of[v] = fill[t]
        fill[t] += 1
        load[t] += cnt[v]

    core_of = bin_of // ST
    st_of = bin_of % ST

    ec = core_of[dst]
    est = st_of[dst]
    gid = (ec.astype(np.int64) * ST + est) * NBANK + sb
    NG = NCORE * ST * NBANK
    eorder = np.argsort(gid, kind="stable")
    gid_s = gid[eorder]
    src_s = src[eorder]
    dst_s = dst[eorder]
    sizes = np.bincount(gid_s, minlength=NG)
    starts = np.zeros(NG + 1, np.int64)
    np.cumsum(sizes, out=starts[1:])
    rank = np.arange(len(gid_s), dtype=np.int64) - starts[gid_s]

    C_B = int(-(-sizes.max() // P))
    C4 = C_B * NBANK
    CAP = C_B * P

    idx_local = np.zeros((NCORE, ST, NBANK, CAP), np.int16)
    dloc_all = np.full((NCORE, ST, NBANK, CAP), 300.0, np.float32)
    g_core = (gid_s // (ST * NBANK)).astype(np.int64)
    g_st = (gid_s // NBANK) % ST
    g_b = gid_s % NBANK
    idx_local[g_core, g_st, g_b, rank] = (src_s % BANK).astype(np.int16)
    dloc_all[g_core, g_st, g_b, rank] = l_of[dst_s].astype(np.float32)

    # dma_gather index layout: slot s -> [s % 16, s // 16], replicated x8
    il = idx_local.reshape(NCORE, ST, NBANK, CAP // 16, 16)
    il = np.moveaxis(il, -1, -2).reshape(NCORE, ST, NBANK * 16, CAP // 16)
    idxs_host = np.zeros((NCORE, ST, P, NBANK * CAP // 16), np.int16)
    for b in range(NBANK):
        blk = il[:, :, b * 16:(b + 1) * 16, :]
        idxs_host[:, :, :, b * (CAP // 16):(b + 1) * (CAP // 16)] = np.tile(blk, (1, 1, 8, 1))

    # dloc layout: chunk j = b*C_B + r//128 at column st*C4 + j, partition r%128
    dl = dloc_all.reshape(NCORE, ST, NBANK, C_B, P)
    dl = np.moveaxis(dl, -1, -3)
    dloc_host = dl.reshape(NCORE, ST, P, C4).transpose(0, 2, 1, 3).reshape(NCORE, P, ST * C4)

    slot_node = np.full((NCORE, NV), -1, np.int64)
    slot_node[core_of, st_of * NDEST + l_of] = np.arange(N)
    xT_host = np.zeros((NCORE, D, NV), np.float32)
    dinvrep_host = np.zeros((NCORE, D, NV), np.float32)
    for c in range(NCORE):
        m = slot_node[c] >= 0
        xT_host[c][:, m] = x[slot_node[c][m]].T
        dinvrep_host[c][:, m] = np.broadcast_to(dinv[slot_node[c][m]], (D, int(m.sum())))

    iota = np.broadcast_to(np.arange(NDEST, dtype=ml_dtypes.bfloat16), (P, NDEST)).copy()
    meta = dict(N=N, NV=NV, ST=ST, C_B=C_B, C4=C4, CAP=CAP, BANK=BANK)
    shared = dict(ypack=ypack, iota=iota)
    per_core = []
    for c in range(NCORE):
        per_core.append(dict(
            idxs=np.ascontiguousarray(idxs_host[c].reshape(ST * P, NBANK * CAP // 16)),
            dloc=np.ascontiguousarray(dloc_host[c]),
            xT=np.ascontiguousarray(xT_host[c]),
            dinvrep=np.ascontiguousarray(dinvrep_host[c]),
        ))
    return meta, shared, per_core, slot_node


def _build_kernel(meta):
    import concourse.bacc as bacc
    import concourse.tile as tile
    from concourse import mybir
    from concourse.library_config import mlp

    N, NV, ST, C_B, C4, CAP, BANK = (meta[k] for k in
                                     ("N", "NV", "ST", "C_B", "C4", "CAP", "BANK"))
    IDXW = NBANK * CAP // 16
    f32, f32r, bf16, i16 = (mybir.dt.float32, mybir.dt.float32r,
                            mybir.dt.bfloat16, mybir.dt.int16)
    AT = mybir.ActivationFunctionType
    OP = mybir.AluOpType

    nc = bacc.Bacc("TRN2", target_bir_lowering=False, debug=False, num_devices=NCORE)
    t_y = nc.dram_tensor("ypack", [NBANK * BANK, 2 * D], bf16, kind="ExternalInput")
    t_W = nc.dram_tensor("W", [D, D], f32r, kind="ExternalInput")
    t_iota = nc.dram_tensor("iota", [P, NDEST], bf16, kind="ExternalInput")
    t_gamma = nc.dram_tensor("gamma", [D, 1], f32, kind="ExternalInput")
    t_beta = nc.dram_tensor("beta", [D, 1], f32, kind="ExternalInput")
    t_idxs = nc.dram_tensor("idxs", [ST * P, IDXW], i16, kind="ExternalInput")
    t_dloc = nc.dram_tensor("dloc", [P, ST * C4], f32, kind="ExternalInput")
    t_xT = nc.dram_tensor("xT", [D, NV], f32, kind="ExternalInput")
    t_dinv = nc.dram_tensor("dinvrep", [D, NV], f32, kind="ExternalInput")
    o_out = nc.dram_tensor("outT", [D, NV], f32, kind="ExternalOutput")

    with tile.TileContext(nc) as tc:
        with tc.tile_pool(name="const", bufs=1) as cpool, \
             tc.tile_pool(name="sbuf", bufs=3) as sbuf, \
             tc.tile_pool(name="gath", bufs=2) as gpool, \
             tc.tile_pool(name="spool", bufs=4) as spool, \
             tc.tile_pool(name="psum", bufs=2, space="PSUM") as psum, \
             tc.tile_pool(name="dram", bufs=1, space="DRAM") as dram:
            nc.gpsimd.load_library(mlp)
            iota_sb = cpool.tile([P, NDEST], bf16)
            W_sb = cpool.tile([D, D], f32r)
            gamma_sb = cpool.tile([D, 1], f32)
            beta_sb = cpool.tile([D, 1], f32)
            dloc_sb = cpool.tile([P, ST * C4], f32)
            outpre = cpool.tile([D, NV], f32)
            sumcol = cpool.tile([D, ST], f32)
            sqcol = cpool.tile([D, ST], f32)
            nc.sync.dma_start(out=iota_sb[:], in_=t_iota[:])
            nc.sync.dma_start(out=W_sb[:], in_=t_W[:])
            nc.sync.dma_start(out=gamma_sb[:], in_=t_gamma[:])
            nc.sync.dma_start(out=beta_sb[:], in_=t_beta[:])
            nc.sync.dma_start(out=dloc_sb[:], in_=t_dloc[:])

            for st in range(ST):
                idxs_t = sbuf.tile([P, IDXW], i16, name="idxs_t", tag="idxs")
                nc.sync.dma_start(out=idxs_t[:], in_=t_idxs[st * P:(st + 1) * P, :])
                msgs = gpool.tile([P, C4, 2 * D], bf16, name="msgs", tag="msgs")
                for b in range(NBANK):
                    nc.gpsimd.dma_gather(
                        msgs[:, b * C_B:(b + 1) * C_B, :],
                        t_y[b * BANK:(b + 1) * BANK, :],
                        idxs_t[:, b * (CAP // 16):(b + 1) * (CAP // 16)],
                        CAP, CAP, 2 * D,
                        single_packet=False,
                    )
                xT_t = sbuf.tile([D, NDEST], f32, name="xT_t", tag="xT")
                dinv_t = sbuf.tile([D, NDEST], f32, name="dinv_t", tag="dinv")
                nc.sync.dma_start(out=xT_t[:], in_=t_xT[:, st * NDEST:(st + 1) * NDEST])
                nc.sync.dma_start(out=dinv_t[:], in_=t_dinv[:, st * NDEST:(st + 1) * NDEST])

                agg_ps = psum.tile([D, NDEST], f32, space="PSUM", name="agg_ps", tag="agg")
                for j in range(C4):
                    S_t = spool.tile([P, NDEST], bf16, name="S_t", tag="S")
                    nc.vector.tensor_scalar(
                        out=S_t[:], in0=iota_sb[:],
                        scalar1=dloc_sb[:, st * C4 + j:st * C4 + j + 1],
                        scalar2=None, op0=OP.is_equal,
                    )
                    nc.tensor.matmul(out=agg_ps[:], lhsT=msgs[:, j, 0:D], rhs=S_t[:],
                                     start=(j == 0), stop=False)
                    nc.tensor.matmul(out=agg_ps[:], lhsT=msgs[:, j, D:2 * D], rhs=S_t[:],
                                     start=False, stop=(j == C4 - 1))

                agg_sb = sbuf.tile([D, NDEST], f32r, name="agg_sb", tag="aggsb")
                nc.vector.tensor_tensor(out=agg_sb[:], in0=agg_ps[:], in1=dinv_t[:],
                                        op=OP.mult)
                out2_ps = psum.tile([D, NDEST], f32, space="PSUM", name="out2_ps", tag="out2")
                nc.tensor.matmul(out=out2_ps[:], lhsT=W_sb[:], rhs=agg_sb[:],
                                 start=True, stop=True)
                op_slice = outpre[:, st * NDEST:(st + 1) * NDEST]
                nc.vector.tensor_tensor(out=op_slice, in0=out2_ps[:], in1=xT_t[:],
                                        op=OP.add)
                scr = sbuf.tile([D, NDEST], f32, name="scr", tag="scr")
                nc.scalar.activation(out=scr[:], in_=op_slice, func=AT.Copy,
                                     accum_out=sumcol[:, st:st + 1])
                nc.scalar.activation(out=scr[:], in_=op_slice, func=AT.Square,
                                     accum_out=sqcol[:, st:st + 1])

            stats = cpool.tile([D, 2], f32)
            nc.vector.tensor_reduce(out=stats[:, 0:1], in_=sumcol[:],
                                    axis=mybir.AxisListType.X, op=OP.add)
            nc.vector.tensor_reduce(out=stats[:, 1:2], in_=sqcol[:],
                                    axis=mybir.AxisListType.X, op=OP.add)
            cc_in = dram.tile([D, 2], f32)
            cc_out = dram.tile([D, 2], f32, addr_space="Shared")
            nc.sync.dma_start(out=cc_in[:], in_=stats[:])
            nc.gpsimd.collective_compute(
                "AllReduce", OP.add, replica_groups=[list(range(NCORE))],
                ins=[cc_in[:]], outs=[cc_out[:]],
            )
            ar = cpool.tile([D, 2], f32)
            nc.sync.dma_start(out=ar[:], in_=cc_out[:])

            mean = cpool.tile([D, 1], f32)
            ex2 = cpool.tile([D, 1], f32)
            var = cpool.tile([D, 1], f32)
            A_t = cpool.tile([D, 1], f32)
            B_t = cpool.tile([D, 1], f32)
            inv_n = 1.0 / float(N)
            nc.vector.tensor_scalar(out=mean[:], in0=ar[:, 0:1], scalar1=inv_n,
                                    scalar2=None, op0=OP.mult)
            nc.vector.tensor_scalar(out=ex2[:], in0=ar[:, 1:2], scalar1=inv_n,
                                    scalar2=None, op0=OP.mult)
            m2 = cpool.tile([D, 1], f32)
            nc.vector.tensor_tensor(out=m2[:], in0=mean[:], in1=mean[:], op=OP.mult)
            nc.vector.tensor_tensor(out=var[:], in0=ex2[:], in1=m2[:], op=OP.subtract)
            varp = cpool.tile([D, 1], f32)
            nc.vector.tensor_scalar(out=varp[:], in0=var[:], scalar1=BN_EPS,
                                    scalar2=None, op0=OP.add)
            sdev = cpool.tile([D, 1], f32)
            nc.scalar.activation(out=sdev[:], in_=varp[:], func=AT.Sqrt)
            rstd = cpool.tile([D, 1], f32)
            nc.vector.reciprocal(out=rstd[:], in_=sdev[:])
            nc.vector.tensor_tensor(out=A_t[:], in0=rstd[:], in1=gamma_sb[:], op=OP.mult)
            mA = cpool.tile([D, 1], f32)
            nc.vector.tensor_tensor(out=mA[:], in0=mean[:], in1=A_t[:], op=OP.mult)
            nc.vector.tensor_tensor(out=B_t[:], in0=beta_sb[:], in1=mA[:], op=OP.subtract)

            for st in range(ST):
                fin = sbuf.tile([D, NDEST], f32, name="fin", tag="fin")
                nc.scalar.activation(out=fin[:], in_=outpre[:, st * NDEST:(st + 1) * NDEST],
                                     func=AT.Relu, bias=B_t[:, 0:1], scale=A_t[:, 0:1])
                nc.sync.dma_start(out=o_out[:, st * NDEST:(st + 1) * NDEST], in_=fin[:])

    nc.compile()
    return nc


def kernel(x, edge_index, W, b, gamma, beta, _trace=False):
    from concourse.bass_utils import run_bass_kernel_spmd
    x = np.asarray(x, dtype=np.float32)
    edge_index = np.asarray(edge_index)
    W = np.ascontiguousarray(np.asarray(W, dtype=np.float32))
    gamma = np.asarray(gamma, dtype=np.float32)
    beta = np.asarray(beta, dtype=np.float32)

    meta, shared, per_core, slot_node = _prepare(x, edge_index)
    nc = _build_kernel(meta)
    shared = dict(shared, W=W,
                  gamma=np.ascontiguousarray(gamma.reshape(D, 1)),
                  beta=np.ascontiguousarray(beta.reshape(D, 1)))
    in_maps = [{**shared, **pc} for pc in per_core]
    res = run_bass_kernel_spmd(nc, in_maps, list(range(NCORE)), trace=_trace)

    N = meta["N"]
    out = np.empty((N, D), np.float32)
    for c in range(NCORE):
        m = slot_node[c] >= 0
        out[slot_node[c][m]] = res.results[c]["outT"].T[m]
    if _trace:
        kernel.last_results = res
    return out

